# revision 15
# baseline (speedup 1.0000x reference)
"""Fused MHA Bass kernel for Trainium2, batch-parallel over 8 cores, bf16.

Reference (per batch element):
    qkv = x @ w_qkv + b_qkv ; q,k,v = split(qkv)
    s = q @ k.T / 8 ; a = softmax(s) ; y = (a @ v) @ w_out + b_out

Structural folding (exact algebra, host-side weight fusion):
    s*8 = x M x^T + 1 (x wk bq)^T + [per-row terms],   M = wq wk^T
    y   = (a_unnorm @ u) / den + (b_out + bv w_out),   u = x (wv w_out)
- The k/v projections and the output projection collapse into two [768,768]
  device matmuls (q' = x@M, u = x@N).
- The per-row (tq) score bias terms cancel under softmax shift-invariance
  and are simply dropped; the per-column (tk) term x@(wk bq) is a
  per-partition bias folded into the q'-eviction for free.
- Attention output is computed directly in [tq, dy] layout (exps stationary),
  so the softmax denominator is an appended ones-column of u, landing
  per-partition: one DVE reciprocal, no transposes anywhere, and the
  normalization + output bias fuse into the PSUM->SBUF y eviction.
Everything runs in bf16 (same PE rate as f32r, half the SBUF/DMA): all
tensors SBUF-resident, no DRAM spills, zero collectives.

Per-core PE work: q'-proj 73.7k + u-proj 73.7k + scores 196.6k +
attn 196.9k = 541k PE columns (baseline structure: 688k) -- essentially
100% PE-bound. Measured: rel err 1.02e-2 (gate 2e-2); HW exec time
192271 ns (same differential-NEFF method as the 324506 ns baseline).
An fp8 DoubleRow variant (hi+lo split operands, 3 cross-terms per matmul)
was built and passed at rel err 8.3e-3, but measured 335 us: on real HW
DoubleRow delivers ~1.0 col/cycle (2x flops via K=256/inst, matching the
157 TF/s spec), not the cost model's 0.5, so 3-term fp8 (9 insts/tile)
loses to exact bf16 (6 insts/tile) everywhere a correction term is needed.
"""

import numpy as np
import ml_dtypes

import concourse.bacc as bacc
import concourse.bass as bass
import concourse.mybir as mybir
import concourse.tile as tile
from concourse import bass_utils

F32 = mybir.dt.float32
BF16 = mybir.dt.bfloat16
AF = mybir.ActivationFunctionType

B = 8
T = 2048
D = 768
ND = D // 128           # 6 d-tiles
NT = T // 128           # 16 t-tiles
TQB = 512               # query-block width
NBLK = T // TQB         # 4 blocks
UW = D + 8              # u width: col D = 1.0 denominator column, rest pad
BFNP = ml_dtypes.bfloat16


def _build_program(nc, reps=1):
    x_d = nc.dram_tensor("xt_bf", [D, T], BF16, kind="ExternalInput").ap()
    m_d = nc.dram_tensor("m_bf", [D, D], BF16, kind="ExternalInput").ap()
    n_d = nc.dram_tensor("n_bf", [D, D], BF16, kind="ExternalInput").ap()
    mvk_d = nc.dram_tensor("mvkt", [128, ND], F32, kind="ExternalInput").ap()
    bo2_d = nc.dram_tensor("bo2", [128, D], F32, kind="ExternalInput").ap()
    y_d = nc.dram_tensor("y", [T, D], F32, kind="ExternalOutput").ap()

    with tile.TileContext(nc) as tc:
        for _ in range(reps):
            _emit(tc, nc, x_d, m_d, n_d, mvk_d, bo2_d, y_d)
    nc.compile()


def _emit(tc, nc, x_d, m_d, n_d, mvk_d, bo2_d, y_d):
    with (
        tc.tile_pool(name="const", bufs=1) as cp,
        tc.tile_pool(name="xw", bufs=1) as xp,
        tc.tile_pool(name="qu", bufs=1) as qp,
        tc.tile_pool(name="ex", bufs=2) as ep,
        tc.tile_pool(name="ps", bufs=4, space="PSUM") as pp,
        tc.tile_pool(name="yev", bufs=3) as yp,
    ):
        xbf = xp.tile([128, ND, T], BF16)
        mbf = xp.tile([128, ND, D], BF16)
        nbf = xp.tile([128, ND, D], BF16)
        mvkt = cp.tile([128, ND], F32)
        bo2 = cp.tile([128, D], F32)
        qbf = qp.tile([128, ND, T], BF16)
        ubf = qp.tile([128, NT, UW], BF16)

        # ---- input DMAs: first q'-proj group needs xbf chunk 0 + mbf ----
        for n in range(NBLK):
            nc.sync.dma_start(
                xbf[:, :, n * TQB:(n + 1) * TQB],
                x_d[:, n * TQB:(n + 1) * TQB].rearrange("(j p) t -> p j t", p=128),
            )
        nc.sync.dma_start(mbf[:], m_d.rearrange("(j p) e -> p j e", p=128))
        nc.sync.dma_start(mvkt[:], mvk_d)
        nc.sync.dma_start(nbf[:], n_d.rearrange("(j p) e -> p j e", p=128))
        nc.sync.dma_start(bo2[:], bo2_d)
        nc.vector.memset(ubf[:, :, D:D + 1], 1.0)  # denominator column

        def emit_qproj(n):
            # PSUM = (x@M)[e-tile m, t-chunk n]; evict bf16 + per-e bias (x wk bq)
            for m in range(ND):
                ps = pp.tile([128, TQB], F32, tag="ps")
                for j in range(ND):
                    nc.tensor.matmul(
                        ps[:], mbf[:, j, m * 128:(m + 1) * 128],
                        xbf[:, j, n * TQB:(n + 1) * TQB],
                        start=(j == 0), stop=(j == ND - 1),
                    )
                nc.scalar.activation(qbf[:, m, n * TQB:(n + 1) * TQB], ps[:],
                                     AF.Identity, bias=mvkt[:, m:m + 1])

        def emit_uproj(i):
            for ch in range(2):
                ps = pp.tile([128, 384], F32, tag="ps")
                for j in range(ND):
                    nc.tensor.matmul(
                        ps[:], xbf[:, j, i * 128:(i + 1) * 128],
                        nbf[:, j, ch * 384:(ch + 1) * 384],
                        start=(j == 0), stop=(j == ND - 1),
                    )
                nc.scalar.activation(ubf[:, i, ch * 384:(ch + 1) * 384], ps[:],
                                     AF.Identity)

        def emit_scores(blk, eb):
            # scores^T tile [tk, tq]; exp(s/8 [+ per-tk bias]) fused into eviction
            tq = slice(blk * TQB, (blk + 1) * TQB)
            for i in range(NT):
                ps = pp.tile([128, TQB], F32, tag="ps")
                for j in range(ND):
                    nc.tensor.matmul(
                        ps[:], xbf[:, j, i * 128:(i + 1) * 128], qbf[:, j, tq],
                        start=(j == 0), stop=(j == ND - 1),
                    )
                nc.scalar.activation(eb[:, i, :], ps[:], AF.Exp, scale=0.125)

        def emit_attn(blk, eb):
            # y[tq, dy] = (e @ u) * recip + bo2, denominator from u's ones-column
            for l in range(TQB // 128):
                g = blk * (TQB // 128) + l
                tq = slice(l * 128, (l + 1) * 128)
                yt = yp.tile([128, D], F32, tag="yt")
                rc = yp.tile([128, 1], F32, tag="rc", bufs=2)
                for ch in (1, 0):  # denominator chunk first
                    lo = ch * 384
                    hi = D + 1 if ch == 1 else 384
                    ps = pp.tile([128, hi - lo], F32, tag="ys", bufs=2)
                    for i in range(NT):
                        nc.tensor.matmul(
                            ps[:], eb[:, i, tq], ubf[:, i, lo:hi],
                            start=(i == 0), stop=(i == NT - 1),
                        )
                    if ch == 1:
                        nc.vector.reciprocal(rc[:], ps[:, D - lo:D - lo + 1])
                        nc.vector.scalar_tensor_tensor(
                            yt[:, lo:D], ps[:, 0:D - lo], rc[:], bo2[:, lo:D],
                            op0=mybir.AluOpType.mult, op1=mybir.AluOpType.add,
                        )
                    else:
                        nc.vector.scalar_tensor_tensor(
                            yt[:, lo:384], ps[:], rc[:], bo2[:, lo:384],
                            op0=mybir.AluOpType.mult, op1=mybir.AluOpType.add,
                        )
                nc.sync.dma_start(y_d[g * 128:(g + 1) * 128, :], yt[:])

        # ---- schedule: interleave so exp/DVE evictions hide under PE ----
        eb = [ep.tile([128, NT, TQB], BF16, tag="ebf", name=f"eb{p}")
              for p in range(2)]

        emit_qproj(0)
        emit_scores(0, eb[0])
        for n in range(1, NBLK):
            emit_qproj(n)
        for i in range(NT):
            emit_uproj(i)
        emit_scores(1, eb[1])
        emit_attn(0, eb[0])
        emit_scores(2, eb[0])
        emit_attn(1, eb[1])
        emit_scores(3, eb[1])
        emit_attn(2, eb[0])
        emit_attn(3, eb[1])


_NC_CACHE = None


def build_nc(reps=1):
    nc = bacc.Bacc("TRN2", target_bir_lowering=False, debug=False)
    _build_program(nc, reps=reps)
    return nc


def _get_nc():
    global _NC_CACHE
    if _NC_CACHE is None:
        _NC_CACHE = build_nc(1)
    return _NC_CACHE


def host_prep(x, w_qkv, b_qkv, w_out, b_out):
    """Host-side weight folding. Returns (shared input dict, per-core xT list)."""
    x = np.asarray(x, np.float32)
    w_qkv = np.asarray(w_qkv, np.float32)
    b_qkv = np.asarray(b_qkv, np.float32)
    w_out = np.asarray(w_out, np.float32)
    b_out = np.asarray(b_out, np.float32)

    wq, wk, wv = w_qkv[:, :D], w_qkv[:, D:2 * D], w_qkv[:, 2 * D:]
    bq, bk, bv = b_qkv[:D], b_qkv[D:2 * D], b_qkv[2 * D:]
    shared = {
        "m_bf": (wq @ wk.T).astype(BFNP),
        "n_bf": (wv @ w_out).astype(BFNP),
        "mvkt": np.ascontiguousarray((wk @ bq).reshape(ND, 128).T.astype(np.float32)),
        "bo2": np.ascontiguousarray(
            np.broadcast_to((b_out + bv @ w_out).reshape(1, D), (128, D))),
    }
    xts = [np.ascontiguousarray(x[c].T).astype(BFNP) for c in range(x.shape[0])]
    return shared, xts


def kernel(x, w_qkv, b_qkv, w_out, b_out):
    shared, xts = host_prep(x, w_qkv, b_qkv, w_out, b_out)
    nc = _get_nc()
    in_maps = [{**shared, "xt_bf": xts[c]} for c in range(B)]
    try:
        res = bass_utils.run_bass_kernel_spmd(nc, in_maps, core_ids=list(range(B)))
    except Exception:
        res = bass_utils.run_bass_kernel_spmd(nc, in_maps, core_ids=list(range(B)))
    return np.stack([res.results[c]["y"] for c in range(B)], axis=0)


# revision 17
# speedup vs baseline: 1.6663x; 1.6663x over previous
"""Fused MHA Bass kernel for Trainium2, batch-parallel over 8 cores, bf16.

Reference (per batch element):
    qkv = x @ w_qkv + b_qkv ; q,k,v = split(qkv)
    s = q @ k.T / 8 ; a = softmax(s) ; y = (a @ v) @ w_out + b_out

Structural folding (exact algebra, host-side weight fusion):
    s*8 = x M x^T + 1 (x wk bq)^T + [per-row terms],   M = wq wk^T
    y   = (a_unnorm @ u) / den + (b_out + bv w_out),   u = x (wv w_out)
- The k/v projections and the output projection collapse into two [768,768]
  device matmuls (q' = x@M, u = x@N).
- The per-row (tq) score bias terms cancel under softmax shift-invariance
  and are simply dropped; the per-column (tk) term x@(wk bq) is a
  per-partition bias folded into the q'-eviction for free.
- Attention output is computed directly in [tq, dy] layout (exps stationary),
  so the softmax denominator is an appended ones-column of u, landing
  per-partition: one DVE reciprocal, no transposes anywhere, and the
  normalization + output bias fuse into the PSUM->SBUF y eviction.
Everything runs in bf16 (same PE rate as f32r, half the SBUF/DMA): all
tensors SBUF-resident, no DRAM spills, zero collectives.

Per-core PE work: q'-proj 73.7k + u-proj 73.7k + scores 196.6k +
attn 196.9k = 541k PE columns (baseline structure: 688k) -- essentially
100% PE-bound. Measured: rel err 1.02e-2 (gate 2e-2); HW exec time
192271 ns (same differential-NEFF method as the 324506 ns baseline).
An fp8 DoubleRow variant (hi+lo split operands, 3 cross-terms per matmul)
was built and passed at rel err 8.3e-3, but measured 335 us: on real HW
DoubleRow delivers ~1.0 col/cycle (2x flops via K=256/inst, matching the
157 TF/s spec), not the cost model's 0.5, so 3-term fp8 (9 insts/tile)
loses to exact bf16 (6 insts/tile) everywhere a correction term is needed.
"""

import numpy as np
import ml_dtypes

import concourse.bacc as bacc
import concourse.bass as bass
import concourse.mybir as mybir
import concourse.tile as tile
from concourse import bass_utils

F32 = mybir.dt.float32
BF16 = mybir.dt.bfloat16
AF = mybir.ActivationFunctionType

B = 8
T = 2048
D = 768
ND = D // 128           # 6 d-tiles
NT = T // 128           # 16 t-tiles
TQB = 512               # query-block width
NBLK = T // TQB         # 4 blocks
UW = D + 8              # u width: col D = 1.0 denominator column, rest pad
BFNP = ml_dtypes.bfloat16


def _build_program(nc, reps=1):
    x_d = nc.dram_tensor("xt_bf", [D, T], BF16, kind="ExternalInput").ap()
    m_d = nc.dram_tensor("m_bf", [D, D], BF16, kind="ExternalInput").ap()
    n_d = nc.dram_tensor("n_bf", [D, D], BF16, kind="ExternalInput").ap()
    mvk_d = nc.dram_tensor("mvkt", [128, ND], F32, kind="ExternalInput").ap()
    bo2_d = nc.dram_tensor("bo2", [128, D], F32, kind="ExternalInput").ap()
    y_d = nc.dram_tensor("y", [T, D], F32, kind="ExternalOutput").ap()

    with tile.TileContext(nc) as tc:
        for _ in range(reps):
            _emit(tc, nc, x_d, m_d, n_d, mvk_d, bo2_d, y_d)
    nc.compile()


def _emit(tc, nc, x_d, m_d, n_d, mvk_d, bo2_d, y_d):
    with (
        tc.tile_pool(name="const", bufs=1) as cp,
        tc.tile_pool(name="xw", bufs=1) as xp,
        tc.tile_pool(name="qu", bufs=1) as qp,
        tc.tile_pool(name="ex", bufs=2) as ep,
        tc.tile_pool(name="ps", bufs=6, space="PSUM") as pp,
        tc.tile_pool(name="yev", bufs=3) as yp,
    ):
        xbf = xp.tile([128, ND, T], BF16)
        mbf = xp.tile([128, ND, D], BF16)
        nbf = xp.tile([128, ND, D], BF16)
        mvkt = cp.tile([128, ND], F32)
        bo2 = cp.tile([128, D], F32)
        qbf = qp.tile([128, ND, T], BF16)
        ubf = qp.tile([128, NT, UW], BF16)

        # ---- input DMAs: first q'-proj group needs xbf chunk 0 + mbf ----
        for n in range(NBLK):
            nc.sync.dma_start(
                xbf[:, :, n * TQB:(n + 1) * TQB],
                x_d[:, n * TQB:(n + 1) * TQB].rearrange("(j p) t -> p j t", p=128),
            )
        # split so the first q'-proj group only waits for the first third of M
        nc.sync.dma_start(mbf[:, :, 0:256],
                          m_d[:, 0:256].rearrange("(j p) e -> p j e", p=128))
        nc.sync.dma_start(mbf[:, :, 256:D],
                          m_d[:, 256:D].rearrange("(j p) e -> p j e", p=128))
        nc.sync.dma_start(mvkt[:], mvk_d)
        nc.sync.dma_start(nbf[:], n_d.rearrange("(j p) e -> p j e", p=128))
        nc.sync.dma_start(bo2[:], bo2_d)
        nc.vector.memset(ubf[:, :, D:D + 1], 1.0)  # denominator column

        def emit_qproj(n):
            # PSUM = (x@M)[e-tile m, t-chunk n]; evict bf16 + per-e bias (x wk bq)
            for m in range(ND):
                ps = pp.tile([128, TQB], F32, tag="ps")
                for j in range(ND):
                    nc.tensor.matmul(
                        ps[:], mbf[:, j, m * 128:(m + 1) * 128],
                        xbf[:, j, n * TQB:(n + 1) * TQB],
                        start=(j == 0), stop=(j == ND - 1),
                    )
                nc.scalar.activation(qbf[:, m, n * TQB:(n + 1) * TQB], ps[:],
                                     AF.Identity, bias=mvkt[:, m:m + 1])

        def emit_uproj(i):
            for ch in range(2):
                ps = pp.tile([128, 384], F32, tag="ps")
                for j in range(ND):
                    nc.tensor.matmul(
                        ps[:], xbf[:, j, i * 128:(i + 1) * 128],
                        nbf[:, j, ch * 384:(ch + 1) * 384],
                        start=(j == 0), stop=(j == ND - 1),
                    )
                nc.scalar.activation(ubf[:, i, ch * 384:(ch + 1) * 384], ps[:],
                                     AF.Identity)

        def emit_scores(blk, eb):
            # scores^T tile [tk, tq]; exp(s/8 [+ per-tk bias]) fused into eviction
            tq = slice(blk * TQB, (blk + 1) * TQB)
            for i in range(NT):
                ps = pp.tile([128, TQB], F32, tag="ps")
                for j in range(ND):
                    nc.tensor.matmul(
                        ps[:], xbf[:, j, i * 128:(i + 1) * 128], qbf[:, j, tq],
                        start=(j == 0), stop=(j == ND - 1),
                    )
                nc.scalar.activation(eb[:, i, :], ps[:], AF.Exp, scale=0.125)

        def emit_attn(blk, eb):
            # y[tq, dy] = (e @ u) * recip + bo2, denominator from u's ones-column
            for l in range(TQB // 128):
                g = blk * (TQB // 128) + l
                tq = slice(l * 128, (l + 1) * 128)
                yt = yp.tile([128, D], F32, tag="yt")
                rc = yp.tile([128, 1], F32, tag="rc", bufs=2)
                for ch in (1, 0):  # denominator chunk first
                    lo = ch * 384
                    hi = D + 1 if ch == 1 else 384
                    ps = pp.tile([128, hi - lo], F32, tag="ys", bufs=2)
                    for i in range(NT):
                        nc.tensor.matmul(
                            ps[:], eb[:, i, tq], ubf[:, i, lo:hi],
                            start=(i == 0), stop=(i == NT - 1),
                        )
                    if ch == 1:
                        nc.vector.reciprocal(rc[:], ps[:, D - lo:D - lo + 1])
                        nc.vector.scalar_tensor_tensor(
                            yt[:, lo:D], ps[:, 0:D - lo], rc[:], bo2[:, lo:D],
                            op0=mybir.AluOpType.mult, op1=mybir.AluOpType.add,
                        )
                    else:
                        nc.vector.scalar_tensor_tensor(
                            yt[:, lo:384], ps[:], rc[:], bo2[:, lo:384],
                            op0=mybir.AluOpType.mult, op1=mybir.AluOpType.add,
                        )
                nc.sync.dma_start(y_d[g * 128:(g + 1) * 128, :], yt[:])

        # ---- schedule: interleave so exp/DVE evictions hide under PE ----
        eb = [ep.tile([128, NT, TQB], BF16, tag="ebf", name=f"eb{p}")
              for p in range(2)]

        emit_qproj(0)
        emit_scores(0, eb[0])
        for n in range(1, NBLK):
            emit_qproj(n)
        for i in range(NT):
            emit_uproj(i)
        emit_scores(1, eb[1])
        emit_attn(0, eb[0])
        emit_scores(2, eb[0])
        emit_attn(1, eb[1])
        emit_scores(3, eb[1])
        emit_attn(2, eb[0])
        emit_attn(3, eb[1])


_NC_CACHE = None


def build_nc(reps=1):
    nc = bacc.Bacc("TRN2", target_bir_lowering=False, debug=False)
    _build_program(nc, reps=reps)
    return nc


def _get_nc():
    global _NC_CACHE
    if _NC_CACHE is None:
        _NC_CACHE = build_nc(1)
    return _NC_CACHE


def host_prep(x, w_qkv, b_qkv, w_out, b_out):
    """Host-side weight folding. Returns (shared input dict, per-core xT list)."""
    x = np.asarray(x, np.float32)
    w_qkv = np.asarray(w_qkv, np.float32)
    b_qkv = np.asarray(b_qkv, np.float32)
    w_out = np.asarray(w_out, np.float32)
    b_out = np.asarray(b_out, np.float32)

    wq, wk, wv = w_qkv[:, :D], w_qkv[:, D:2 * D], w_qkv[:, 2 * D:]
    bq, bk, bv = b_qkv[:D], b_qkv[D:2 * D], b_qkv[2 * D:]
    shared = {
        "m_bf": (wq @ wk.T).astype(BFNP),
        "n_bf": (wv @ w_out).astype(BFNP),
        "mvkt": np.ascontiguousarray((wk @ bq).reshape(ND, 128).T.astype(np.float32)),
        "bo2": np.ascontiguousarray(
            np.broadcast_to((b_out + bv @ w_out).reshape(1, D), (128, D))),
    }
    xts = [np.ascontiguousarray(x[c].T).astype(BFNP) for c in range(x.shape[0])]
    return shared, xts


def kernel(x, w_qkv, b_qkv, w_out, b_out):
    shared, xts = host_prep(x, w_qkv, b_qkv, w_out, b_out)
    nc = _get_nc()
    in_maps = [{**shared, "xt_bf": xts[c]} for c in range(B)]
    try:
        res = bass_utils.run_bass_kernel_spmd(nc, in_maps, core_ids=list(range(B)))
    except Exception:
        res = bass_utils.run_bass_kernel_spmd(nc, in_maps, core_ids=list(range(B)))
    return np.stack([res.results[c]["y"] for c in range(B)], axis=0)
